# revision 22
# baseline (speedup 1.0000x reference)
"""MultiHeadRelativeAttention (Transformer-XL style) on 8 Trainium2 NeuronCores.

Sharding: 2 batches x 4 heads per core (core c -> batches {2*(c//4), 2*(c//4)+1},
heads [4*(c%4), 4*(c%4)+4)). Each core computes a partial out-projection over its
256 input channels for each of its 2 batches; the host sums 4 partials per batch.

Pipeline (vs. the previous version):
  - The Transformer-XL relative shift stays a sheared SBUF->SBUF DMA
    (access pattern [[BAND-1,128],[1,1024]] offset 127 over a [128,BAND] band).
  - pos+content add moved off the PE (was an identity matmul) onto Pool/DVE.
  - softmax normalization deferred: exp(A) flows unnormalized into attn@V;
    1/den is applied per-column during the PSUM->SBUF eviction of attn@V
    using a transposed+broadcast reciprocal tile (xbar transpose of packed recs).
  - attn@V matmuls of head-unit u are interleaved into the score matmuls of
    unit u+1 so the PE never waits on the shear/exp/transpose chain.
  - k_pos projection computed once per core for 4 heads (halved vs 8-head shard).
"""
import sys

sys.path.insert(0, '/opt/trn_rl_repo')

import numpy as np
import ml_dtypes

S = 1024          # seq len (query == key)
B = 4             # batch
E = 1024          # embed dim
H = 16            # total heads
D = 64            # head dim
NB = 2            # batches per core
NH = 4            # heads per core
PAIRS = NH // 2   # head pairs per core (2 heads packed per 128 partitions)
CH = NH * D       # channels per core (256)
MT = 2048         # padded positional length (2*S-1 = 2047 valid)
BAND = S + 128    # per-q-tile band width in m
QT = S // 128     # q tiles
KC = S // 128     # k chunks of 128
EC = E // 128     # embed chunks
SCALING = D ** -0.5
N_CORES = 8

_cache = {}


def _build():
    import concourse.bass as bass
    from concourse import bacc
    import concourse.mybir as mybir
    from concourse.tile import TileContext

    bf16 = mybir.dt.bfloat16
    f32 = mybir.dt.float32
    Exp = mybir.ActivationFunctionType.Exp
    ADD = mybir.AluOpType.add
    MULT = mybir.AluOpType.mult

    nc = bacc.Bacc("TRN2", debug=False, num_devices=N_CORES)

    def din(name, shape, dt=bf16):
        return nc.dram_tensor(name, shape, dt, kind='ExternalInput')

    qT = [din(f'qT{b}', [E, S]) for b in range(NB)]
    kT = [din(f'kT{b}', [E, S]) for b in range(NB)]
    vT = [din(f'vT{b}', [E, S]) for b in range(NB)]
    peT = din('peT', [E, MT])
    wqT = din('wqT', [E, CH])
    wkT = din('wkT', [E, CH])
    wvT = din('wvT', [E, CH])
    wpT = din('wpT', [E, CH])
    woT = din('woT', [CH, E])
    cbv = din('cb', [CH, 1], f32)
    pbv = din('pb', [CH, 1], f32)
    outT = [nc.dram_tensor(f'outT{b}', [E, S], f32, kind='ExternalOutput')
            for b in range(NB)]

    with TileContext(nc) as tc:
        with tc.tile_pool(name='persist', bufs=1) as PERS:

            # ---- persistent tiles ----
            qcT = [[PERS.tile([128, S], bf16, name=f'qcT{b}{p}', tag=f'qcT{b}{p}')
                    for p in range(PAIRS)] for b in range(NB)]
            qpT = [[PERS.tile([128, S], bf16, name=f'qpT{b}{p}', tag=f'qpT{b}{p}')
                    for p in range(PAIRS)] for b in range(NB)]
            kTt = [[PERS.tile([128, S], bf16, name=f'kTt{b}{p}', tag=f'kTt{b}{p}')
                    for p in range(PAIRS)] for b in range(NB)]
            kpT = [PERS.tile([128, MT], bf16, name=f'kpT{p}', tag=f'kpT{p}')
                   for p in range(PAIRS)]
            vS = [[PERS.tile([128, CH], bf16, name=f'vS{b}{k}', tag=f'vS{b}{k}')
                   for k in range(KC)] for b in range(NB)]
            woS = [PERS.tile([128, E], bf16, name=f'woS{p}', tag=f'woS{p}')
                   for p in range(PAIRS)]
            # attn@V output (normalized), bf16, [2heads*64d x 512q] per (b,p,sc)
            oT = [[[PERS.tile([128, 512], bf16, name=f'oT{b}{p}{sc}', tag=f'oT{b}{p}{sc}')
                    for sc in range(2)] for p in range(PAIRS)] for b in range(NB)]
            cbS = [PERS.tile([128, 1], f32, name=f'cbS{p}', tag=f'cbS{p}')
                   for p in range(PAIRS)]
            pbS = [PERS.tile([128, 1], f32, name=f'pbS{p}', tag=f'pbS{p}')
                   for p in range(PAIRS)]

            # weight/bias loads on the Act HWDGE queue (SP queue handles the
            # big activations + pe table + transposes)
            for p in range(PAIRS):
                nc.scalar.dma_start(cbS[p][:], cbv.ap()[p * 128:(p + 1) * 128, :])
                nc.scalar.dma_start(pbS[p][:], pbv.ap()[p * 128:(p + 1) * 128, :])
            for p in range(PAIRS):
                nc.scalar.dma_start(woS[p][:], woT.ap()[p * 128:(p + 1) * 128, :])

            # ---- projections ----
            with tc.tile_pool(name='pp', bufs=2, space='PSUM') as PP, \
                 tc.tile_pool(name='stage', bufs=8) as STG:
                win_q = [STG.tile([128, CH], bf16, name='winq', tag='winq') for _ in range(EC)]
                win_k = [STG.tile([128, CH], bf16, name='wink', tag='wink') for _ in range(EC)]
                win_v = [STG.tile([128, CH], bf16, name='winv', tag='winv') for _ in range(EC)]
                win_p = [STG.tile([128, CH], bf16, name='winp', tag='winp') for _ in range(EC)]
                for ec in range(EC):
                    esl = slice(ec * 128, (ec + 1) * 128)
                    nc.scalar.dma_start(win_q[ec][:], wqT.ap()[esl, :])
                    nc.scalar.dma_start(win_k[ec][:], wkT.ap()[esl, :])
                    nc.scalar.dma_start(win_v[ec][:], wvT.ap()[esl, :])
                    nc.scalar.dma_start(win_p[ec][:], wpT.ap()[esl, :])

                # q/k projections per batch -> qcT/qpT (biased) and kTt
                xq, xk, xv = {}, {}, {}
                for b in range(NB):
                    xq[b] = [STG.tile([128, S], bf16, name='xin', tag='xin') for _ in range(EC)]
                    for ec in range(EC):
                        nc.sync.dma_start(xq[b][ec][:], qT[b].ap()[ec * 128:(ec + 1) * 128, :])
                    for p in range(PAIRS):
                        ps = PP.tile([128, S], f32, name='qps', tag='qps')
                        for c in range(2):
                            for ec in range(EC):
                                nc.tensor.matmul(
                                    ps[:, c * 512:(c + 1) * 512],
                                    win_q[ec][:, p * 128:(p + 1) * 128],
                                    xq[b][ec][:, c * 512:(c + 1) * 512],
                                    start=(ec == 0), stop=(ec == EC - 1))
                        nc.vector.tensor_scalar_add(qcT[b][p][:], ps[:], cbS[p][:])
                        nc.vector.tensor_scalar_add(qpT[b][p][:], ps[:], pbS[p][:])

                    xk[b] = [STG.tile([128, S], bf16, name='xin', tag='xin') for _ in range(EC)]
                    for ec in range(EC):
                        nc.sync.dma_start(xk[b][ec][:], kT[b].ap()[ec * 128:(ec + 1) * 128, :])
                    for p in range(PAIRS):
                        ps = PP.tile([128, S], f32, name='qps', tag='qps')
                        for c in range(2):
                            for ec in range(EC):
                                nc.tensor.matmul(
                                    ps[:, c * 512:(c + 1) * 512],
                                    win_k[ec][:, p * 128:(p + 1) * 128],
                                    xk[b][ec][:, c * 512:(c + 1) * 512],
                                    start=(ec == 0), stop=(ec == EC - 1))
                        nc.vector.tensor_copy(kTt[b][p][:], ps[:])

                # k_pos projection -> kpT (once per core, 4 heads)
                for mc in range(MT // 512):
                    pein = [STG.tile([128, 512], bf16, name='pein', tag='pein') for _ in range(EC)]
                    for ec in range(EC):
                        nc.sync.dma_start(
                            pein[ec][:],
                            peT.ap()[ec * 128:(ec + 1) * 128, mc * 512:(mc + 1) * 512])
                    for p in range(PAIRS):
                        ps = PP.tile([128, 512], f32, name='kps', tag='kps')
                        for ec in range(EC):
                            nc.tensor.matmul(
                                ps[:], win_p[ec][:, p * 128:(p + 1) * 128], pein[ec][:],
                                start=(ec == 0), stop=(ec == EC - 1))
                        nc.scalar.copy(kpT[p][:, mc * 512:(mc + 1) * 512], ps[:])

                # v projection -> vS[b][kt] = [128 k x (h,d) 256]
                for b in range(NB):
                    xv[b] = [STG.tile([128, S], bf16, name='xin', tag='xin') for _ in range(EC)]
                    for ec in range(EC):
                        nc.sync.dma_start(xv[b][ec][:], vT[b].ap()[ec * 128:(ec + 1) * 128, :])
                    for kt in range(KC):
                        ps = PP.tile([128, CH], f32, name='vps', tag='vps')
                        for ec in range(EC):
                            nc.tensor.matmul(
                                ps[:], xv[b][ec][:, kt * 128:(kt + 1) * 128], win_v[ec][:],
                                start=(ec == 0), stop=(ec == EC - 1))
                        if kt % 2 == 0:
                            nc.vector.tensor_copy(vS[b][kt][:], ps[:])
                        else:
                            nc.scalar.copy(vS[b][kt][:], ps[:])

            # ---- scores + attention ----
            groups = [(b, p, h01) for b in range(NB) for p in range(PAIRS)
                      for h01 in range(2)]
            units = [(b, p) for b in range(NB) for p in range(PAIRS)]

            with tc.tile_pool(name='sc', bufs=2, space='PSUM') as SC, \
                 tc.tile_pool(name='bandp', bufs=3) as BP, \
                 tc.tile_pool(name='shp', bufs=4) as SHP, \
                 tc.tile_pool(name='sump', bufs=3) as SUP, \
                 tc.tile_pool(name='ap', bufs=4) as AP_, \
                 tc.tile_pool(name='atp', bufs=4) as ATP, \
                 tc.tile_pool(name='scp', bufs=4) as SCP:

                at_tiles = {}    # group idx -> at tile
                ops_tile = {}    # unit idx -> attn@V PSUM tile

                def attn_mms(u):
                    """Build the 32 attn@V matmul closures for unit u."""
                    b, p = units[u]
                    ops = SC.tile([128, S], f32, name=f'O{u}', tag='O', bufs=1)
                    ops_tile[u] = ops
                    atg = [at_tiles[2 * u], at_tiles[2 * u + 1]]
                    mms = []
                    for kc in range(KC):
                        for h01 in range(2):
                            hh = 2 * p + h01
                            for sc in range(2):
                                mms.append(lambda kc=kc, h01=h01, hh=hh, sc=sc:
                                    nc.tensor.matmul(
                                        ops[64 * h01:64 * h01 + 64,
                                            512 * sc:512 * sc + 512],
                                        vS[b][kc][:, hh * 64:hh * 64 + 64],
                                        atg[h01][:, kc, 4 * sc:4 * sc + 4, :],
                                        start=(kc == 0), stop=(kc == KC - 1)))
                    return mms

                def emit_ov_evict(u):
                    b, p = units[u]
                    ops = ops_tile[u]
                    nc.vector.tensor_copy(oT[b][p][0][:], ops[:, 0:512])
                    nc.scalar.copy(oT[b][p][1][:], ops[:, 512:1024])

                pend = []    # pending attn@V matmul closures
                pend_i = 0
                for gi, (b, p, h01) in enumerate(groups):
                    rows = slice(64 * h01, 64 * h01 + 64)
                    if gi >= 2 and gi % 2 == 0:
                        pend = attn_mms(gi // 2 - 1)
                        pend_i = 0
                    at_g = ATP.tile([128, KC, QT, 128], bf16, name=f'at{gi}', tag='at')
                    at_tiles[gi] = at_g
                    a_tiles = []
                    for t in range(QT):
                        # interleave 2 attn@V matmuls of the previous unit
                        for _ in range(2):
                            if pend_i < len(pend):
                                pend[pend_i]()
                                pend_i += 1
                        blo = 896 - 128 * t
                        qsl = slice(t * 128, (t + 1) * 128)
                        # pos band matmuls -> 512-wide PSUM chunks, evicted to SBUF
                        pb_t = BP.tile([128, BAND], bf16, name='Pb', tag='Pb')
                        for ci, (c0, n) in enumerate(((0, 512), (512, 512), (1024, 128))):
                            pps = SC.tile([128, 512], f32, name='P', tag='P')
                            nc.tensor.matmul(
                                pps[:, :n],
                                qpT[b][p][rows, qsl],
                                kpT[p][rows, blo + c0:blo + c0 + n],
                                start=True, stop=True)
                            if ci == 0 or (ci == 1 and t % 2 == 1):
                                nc.scalar.copy(pb_t[:, c0:c0 + n], pps[:, :n])
                            else:
                                nc.vector.tensor_copy(pb_t[:, c0:c0 + n], pps[:, :n])
                        # rel-shift: sheared SBUF->SBUF DMA (SWDGE)
                        src = pb_t[:]
                        sheared = src.__replace__(
                            ap=src.ap.__class__([[BAND - 1, 128], [1, S]]),
                            offset=127)
                        sh_t = SHP.tile([128, S], bf16, name='Sh', tag='Sh')
                        nc.gpsimd.dma_start(sh_t[:], sheared)
                        # content scores
                        cps = SC.tile([128, S], f32, name='C', tag='C')
                        for c in range(2):
                            csl = slice(c * 512, (c + 1) * 512)
                            nc.tensor.matmul(
                                cps[:, csl], qcT[b][p][rows, qsl], kTt[b][p][rows, csl],
                                start=True, stop=True)
                        # pos + content add on DVE (only engine with PSUM tensor ops)
                        sum_t = SUP.tile([128, S], f32, name='Su', tag='Su')
                        nc.vector.tensor_tensor(sum_t[:], cps[:], sh_t[:], ADD)
                        a_t = AP_.tile([128, S], bf16, name='A', tag='A', bufs=4)
                        den = SCP.tile([128, 1], f32, name='den', tag='den')
                        nc.scalar.activation(a_t[:], sum_t[:], Exp, accum_out=den[:])
                        rec = SCP.tile([128, 1], f32, name='rec', tag='rec')
                        nc.vector.reciprocal(rec[:], den[:])
                        as_t = AP_.tile([128, S], bf16, name='As', tag='As', bufs=9)
                        nc.gpsimd.tensor_scalar_mul(as_t[:], a_t[:], rec[:])
                        a_tiles.append(as_t)
                    # batched xbar transposes (minimize DMA xbar-mode transitions)
                    for t in range(QT):
                        nc.sync.dma_start_transpose(at_g[:, :, t, :], a_tiles[t])
                    if gi % 2 == 1 and pend:
                        # drain any leftover attn@V matmuls, then evict
                        while pend_i < len(pend):
                            pend[pend_i]()
                            pend_i += 1
                        pend = []
                        emit_ov_evict(gi // 2 - 1)

                # tail attn@V (unit 3) still inside the score PSUM pool scope
                for mm in attn_mms(3):
                    mm()
                emit_ov_evict(3)

            # ---- out projection (scores PSUM pool closed; fresh banks) ----
            with tc.tile_pool(name='op', bufs=3, space='PSUM') as OP, \
                 tc.tile_pool(name='oev', bufs=3) as OEV:
                for b in range(NB):
                    for sc in range(2):
                        for eb in range(EC):
                            po = OP.tile([128, 512], f32, name='OP', tag='OP')
                            for p in range(PAIRS):
                                nc.tensor.matmul(
                                    po[:],
                                    woS[p][:, eb * 128:(eb + 1) * 128],
                                    oT[b][p][sc][:],
                                    start=(p == 0), stop=(p == PAIRS - 1))
                            ev = OEV.tile([128, 512], f32, name='oev', tag='oev')
                            nc.vector.tensor_copy(ev[:], po[:])
                            nc.scalar.dma_start(
                                outT[b].ap()[eb * 128:(eb + 1) * 128,
                                             sc * 512:(sc + 1) * 512],
                                ev[:])

    nc.compile()
    return nc


def _prep_inputs(inputs):
    """Full inputs -> list of per-core input dicts (host-side shard + layout)."""
    bf = ml_dtypes.bfloat16
    q = np.asarray(inputs['query'], np.float32)
    k = np.asarray(inputs['key'], np.float32)
    v = np.asarray(inputs['value'], np.float32)
    pe = np.asarray(inputs['pe'], np.float32)
    w_q = np.asarray(inputs['w_q'], np.float32)
    w_k = np.asarray(inputs['w_k'], np.float32)
    w_v = np.asarray(inputs['w_v'], np.float32)
    w_kp = np.asarray(inputs['w_k_pos'], np.float32)
    cb = np.asarray(inputs['content_bias'], np.float32)
    pb = np.asarray(inputs['pos_bias'], np.float32)
    w_out = np.asarray(inputs['w_out'], np.float32)

    M = 2 * S - 1
    lower = pe.shape[0] // 2 - S + 1
    pe_sl = pe[lower:lower + M]                     # [2047, E]
    peT = np.zeros((E, MT), np.float32)
    peT[:, :M] = pe_sl.T

    qTb = [np.ascontiguousarray(q[:, b, :].T).astype(bf) for b in range(B)]
    kTb = [np.ascontiguousarray(k[:, b, :].T).astype(bf) for b in range(B)]
    vTb = [np.ascontiguousarray(v[:, b, :].T).astype(bf) for b in range(B)]
    peTb = peT.astype(bf)

    in_maps = []
    for c in range(N_CORES):
        bg, hg = divmod(c, 4)
        hs = hg * NH
        ch = slice(hs * D, (hs + NH) * D)           # this core's 256 channels
        m = {
            'peT': peTb,
            'wqT': np.ascontiguousarray((SCALING * w_q[ch, :]).T).astype(bf),
            'wkT': np.ascontiguousarray(w_k[ch, :].T).astype(bf),
            'wvT': np.ascontiguousarray(w_v[ch, :].T).astype(bf),
            'wpT': np.ascontiguousarray(w_kp[ch, :].T).astype(bf),
            'woT': np.ascontiguousarray(w_out[:, ch].T).astype(bf),
            'cb': (SCALING * cb[hs:hs + NH].reshape(CH, 1)).astype(np.float32),
            'pb': (SCALING * pb[hs:hs + NH].reshape(CH, 1)).astype(np.float32),
        }
        for i in range(NB):
            b = 2 * bg + i
            m[f'qT{i}'] = qTb[b]
            m[f'kT{i}'] = kTb[b]
            m[f'vT{i}'] = vTb[b]
        in_maps.append(m)
    return in_maps


def kernel(**inputs):
    from concourse import bass_utils

    if 'nc' not in _cache:
        _cache['nc'] = _build()
    nc = _cache['nc']

    in_maps = _prep_inputs(inputs)
    res = bass_utils.run_bass_kernel_spmd(nc, in_maps, core_ids=list(range(N_CORES)))
    _cache['last_results'] = res

    b_out = np.asarray(inputs['b_out'], np.float32)
    out = np.empty((S, B, E), np.float32)
    for b in range(B):
        bg, i = divmod(b, NB)
        acc = res.results[4 * bg][f'outT{i}'].copy()
        for hg in range(1, 4):
            acc += res.results[4 * bg + hg][f'outT{i}']
        out[:, b, :] = acc.T + b_out
    return out


# revision 24
# speedup vs baseline: 2.5312x; 2.5312x over previous
"""MultiHeadRelativeAttention (Transformer-XL style) on 8 Trainium2 NeuronCores.

Sharding: 2 batches x 4 heads per core (core c -> batches {2*(c//4), 2*(c//4)+1},
heads [4*(c%4), 4*(c%4)+4)). Each core computes a partial out-projection over its
256 input channels for each of its 2 batches; the host sums 4 partials per batch.

Pipeline (vs. the previous version):
  - The Transformer-XL relative shift stays a sheared SBUF->SBUF DMA
    (access pattern [[BAND-1,128],[1,1024]] offset 127 over a [128,BAND] band).
  - pos+content add moved off the PE (was an identity matmul) onto Pool/DVE.
  - softmax normalization deferred: exp(A) flows unnormalized into attn@V;
    1/den is applied per-column during the PSUM->SBUF eviction of attn@V
    using a transposed+broadcast reciprocal tile (xbar transpose of packed recs).
  - attn@V matmuls of head-unit u are interleaved into the score matmuls of
    unit u+1 so the PE never waits on the shear/exp/transpose chain.
  - k_pos projection computed once per core for 4 heads (halved vs 8-head shard).
"""
import sys

sys.path.insert(0, '/opt/trn_rl_repo')

import numpy as np
import ml_dtypes

S = 1024          # seq len (query == key)
B = 4             # batch
E = 1024          # embed dim
H = 16            # total heads
D = 64            # head dim
NB = 2            # batches per core
NH = 4            # heads per core
PAIRS = NH // 2   # head pairs per core (2 heads packed per 128 partitions)
CH = NH * D       # channels per core (256)
MT = 2048         # padded positional length (2*S-1 = 2047 valid)
BAND = S + 128    # per-q-tile band width in m
QT = S // 128     # q tiles
KC = S // 128     # k chunks of 128
EC = E // 128     # embed chunks
SCALING = D ** -0.5
N_CORES = 8

_cache = {}


def _build():
    import concourse.bass as bass
    from concourse import bacc
    import concourse.mybir as mybir
    from concourse.tile import TileContext

    bf16 = mybir.dt.bfloat16
    f32 = mybir.dt.float32
    Exp = mybir.ActivationFunctionType.Exp
    ADD = mybir.AluOpType.add
    MULT = mybir.AluOpType.mult

    nc = bacc.Bacc("TRN2", debug=False, num_devices=N_CORES)

    def din(name, shape, dt=bf16):
        return nc.dram_tensor(name, shape, dt, kind='ExternalInput')

    qT = [din(f'qT{b}', [E, S]) for b in range(NB)]
    kT = [din(f'kT{b}', [E, S]) for b in range(NB)]
    vT = [din(f'vT{b}', [E, S]) for b in range(NB)]
    peT = din('peT', [E, MT])
    wqT = din('wqT', [E, CH])
    wkT = din('wkT', [E, CH])
    wvT = din('wvT', [E, CH])
    wpT = din('wpT', [E, CH])
    woT = din('woT', [CH, E])
    cbv = din('cb', [CH, 1], f32)
    pbv = din('pb', [CH, 1], f32)
    outT = [nc.dram_tensor(f'outT{b}', [E, S], f32, kind='ExternalOutput')
            for b in range(NB)]

    with TileContext(nc) as tc:
        with tc.tile_pool(name='persist', bufs=1) as PERS:

            # ---- persistent tiles ----
            qcT = [[PERS.tile([128, S], bf16, name=f'qcT{b}{p}', tag=f'qcT{b}{p}')
                    for p in range(PAIRS)] for b in range(NB)]
            qpT = [[PERS.tile([128, S], bf16, name=f'qpT{b}{p}', tag=f'qpT{b}{p}')
                    for p in range(PAIRS)] for b in range(NB)]
            kTt = [[PERS.tile([128, S], bf16, name=f'kTt{b}{p}', tag=f'kTt{b}{p}')
                    for p in range(PAIRS)] for b in range(NB)]
            kpT = [PERS.tile([128, MT], bf16, name=f'kpT{p}', tag=f'kpT{p}')
                   for p in range(PAIRS)]
            vS = [[PERS.tile([128, CH], bf16, name=f'vS{b}{k}', tag=f'vS{b}{k}')
                   for k in range(KC)] for b in range(NB)]
            woS = [PERS.tile([128, E], bf16, name=f'woS{p}', tag=f'woS{p}')
                   for p in range(PAIRS)]
            # attn@V output (normalized), bf16, [2heads*64d x 512q] per (b,p,sc)
            oT = [[[PERS.tile([128, 512], bf16, name=f'oT{b}{p}{sc}', tag=f'oT{b}{p}{sc}')
                    for sc in range(2)] for p in range(PAIRS)] for b in range(NB)]
            cbS = [PERS.tile([128, 1], f32, name=f'cbS{p}', tag=f'cbS{p}')
                   for p in range(PAIRS)]
            pbS = [PERS.tile([128, 1], f32, name=f'pbS{p}', tag=f'pbS{p}')
                   for p in range(PAIRS)]

            # weight/bias loads on the Act HWDGE queue (SP queue handles the
            # big activations + pe table + transposes)
            for p in range(PAIRS):
                nc.scalar.dma_start(cbS[p][:], cbv.ap()[p * 128:(p + 1) * 128, :])
                nc.scalar.dma_start(pbS[p][:], pbv.ap()[p * 128:(p + 1) * 128, :])
            for p in range(PAIRS):
                nc.scalar.dma_start(woS[p][:], woT.ap()[p * 128:(p + 1) * 128, :])

            # ---- projections ----
            with tc.tile_pool(name='pp', bufs=2, space='PSUM') as PP, \
                 tc.tile_pool(name='stage', bufs=8) as STG:
                win_q = [STG.tile([128, CH], bf16, name='winq', tag='winq') for _ in range(EC)]
                win_k = [STG.tile([128, CH], bf16, name='wink', tag='wink') for _ in range(EC)]
                win_v = [STG.tile([128, CH], bf16, name='winv', tag='winv') for _ in range(EC)]
                win_p = [STG.tile([128, CH], bf16, name='winp', tag='winp') for _ in range(EC)]
                for ec in range(EC):
                    esl = slice(ec * 128, (ec + 1) * 128)
                    nc.scalar.dma_start(win_q[ec][:], wqT.ap()[esl, :])
                    nc.scalar.dma_start(win_k[ec][:], wkT.ap()[esl, :])
                    nc.scalar.dma_start(win_v[ec][:], wvT.ap()[esl, :])
                    nc.scalar.dma_start(win_p[ec][:], wpT.ap()[esl, :])

                # q/k projections per batch -> qcT/qpT (biased) and kTt
                xq, xk, xv = {}, {}, {}
                for b in range(NB):
                    xq[b] = [STG.tile([128, S], bf16, name='xin', tag='xin') for _ in range(EC)]
                    for ec in range(EC):
                        nc.sync.dma_start(xq[b][ec][:], qT[b].ap()[ec * 128:(ec + 1) * 128, :])
                    for p in range(PAIRS):
                        ps = PP.tile([128, S], f32, name='qps', tag='qps')
                        for c in range(2):
                            for ec in range(EC):
                                nc.tensor.matmul(
                                    ps[:, c * 512:(c + 1) * 512],
                                    win_q[ec][:, p * 128:(p + 1) * 128],
                                    xq[b][ec][:, c * 512:(c + 1) * 512],
                                    start=(ec == 0), stop=(ec == EC - 1))
                        nc.vector.tensor_scalar_add(qcT[b][p][:], ps[:], cbS[p][:])
                        nc.vector.tensor_scalar_add(qpT[b][p][:], ps[:], pbS[p][:])

                    xk[b] = [STG.tile([128, S], bf16, name='xin', tag='xin') for _ in range(EC)]
                    for ec in range(EC):
                        nc.sync.dma_start(xk[b][ec][:], kT[b].ap()[ec * 128:(ec + 1) * 128, :])
                    for p in range(PAIRS):
                        ps = PP.tile([128, S], f32, name='qps', tag='qps')
                        for c in range(2):
                            for ec in range(EC):
                                nc.tensor.matmul(
                                    ps[:, c * 512:(c + 1) * 512],
                                    win_k[ec][:, p * 128:(p + 1) * 128],
                                    xk[b][ec][:, c * 512:(c + 1) * 512],
                                    start=(ec == 0), stop=(ec == EC - 1))
                        nc.vector.tensor_copy(kTt[b][p][:], ps[:])

                # k_pos projection -> kpT (once per core, 4 heads)
                for mc in range(MT // 512):
                    pein = [STG.tile([128, 512], bf16, name='pein', tag='pein') for _ in range(EC)]
                    for ec in range(EC):
                        nc.sync.dma_start(
                            pein[ec][:],
                            peT.ap()[ec * 128:(ec + 1) * 128, mc * 512:(mc + 1) * 512])
                    for p in range(PAIRS):
                        ps = PP.tile([128, 512], f32, name='kps', tag='kps')
                        for ec in range(EC):
                            nc.tensor.matmul(
                                ps[:], win_p[ec][:, p * 128:(p + 1) * 128], pein[ec][:],
                                start=(ec == 0), stop=(ec == EC - 1))
                        nc.scalar.copy(kpT[p][:, mc * 512:(mc + 1) * 512], ps[:])

                # v projection -> vS[b][kt] = [128 k x (h,d) 256]
                for b in range(NB):
                    xv[b] = [STG.tile([128, S], bf16, name='xin', tag='xin') for _ in range(EC)]
                    for ec in range(EC):
                        nc.sync.dma_start(xv[b][ec][:], vT[b].ap()[ec * 128:(ec + 1) * 128, :])
                    for kt in range(KC):
                        ps = PP.tile([128, CH], f32, name='vps', tag='vps')
                        for ec in range(EC):
                            nc.tensor.matmul(
                                ps[:], xv[b][ec][:, kt * 128:(kt + 1) * 128], win_v[ec][:],
                                start=(ec == 0), stop=(ec == EC - 1))
                        if kt % 2 == 0:
                            nc.vector.tensor_copy(vS[b][kt][:], ps[:])
                        else:
                            nc.scalar.copy(vS[b][kt][:], ps[:])

            # ---- scores + attention ----
            groups = [(b, p, h01) for b in range(NB) for p in range(PAIRS)
                      for h01 in range(2)]
            units = [(b, p) for b in range(NB) for p in range(PAIRS)]

            with tc.tile_pool(name='sc', bufs=2, space='PSUM') as SC, \
                 tc.tile_pool(name='bandp', bufs=3) as BP, \
                 tc.tile_pool(name='shp', bufs=4) as SHP, \
                 tc.tile_pool(name='sump', bufs=3) as SUP, \
                 tc.tile_pool(name='ap', bufs=4) as AP_, \
                 tc.tile_pool(name='atp', bufs=4) as ATP, \
                 tc.tile_pool(name='scp', bufs=4) as SCP:

                at_tiles = {}    # group idx -> at tile
                ops_tile = {}    # unit idx -> attn@V PSUM tile

                def attn_mms(u):
                    """Build the 32 attn@V matmul closures for unit u."""
                    b, p = units[u]
                    ops = SC.tile([128, S], f32, name=f'O{u}', tag='O', bufs=1)
                    ops_tile[u] = ops
                    atg = [at_tiles[2 * u], at_tiles[2 * u + 1]]
                    mms = []
                    for kc in range(KC):
                        for h01 in range(2):
                            hh = 2 * p + h01
                            for sc in range(2):
                                mms.append(lambda kc=kc, h01=h01, hh=hh, sc=sc:
                                    nc.tensor.matmul(
                                        ops[64 * h01:64 * h01 + 64,
                                            512 * sc:512 * sc + 512],
                                        vS[b][kc][:, hh * 64:hh * 64 + 64],
                                        atg[h01][:, kc, 4 * sc:4 * sc + 4, :],
                                        start=(kc == 0), stop=(kc == KC - 1)))
                    return mms

                def emit_ov_evict(u):
                    b, p = units[u]
                    ops = ops_tile[u]
                    nc.vector.tensor_copy(oT[b][p][0][:], ops[:, 0:512])
                    nc.scalar.copy(oT[b][p][1][:], ops[:, 512:1024])

                pend = []    # pending attn@V matmul closures
                pend_i = 0
                for gi, (b, p, h01) in enumerate(groups):
                    rows = slice(64 * h01, 64 * h01 + 64)
                    if gi >= 2 and gi % 2 == 0:
                        pend = attn_mms(gi // 2 - 1)
                        pend_i = 0
                    at_g = ATP.tile([128, KC, QT, 128], bf16, name=f'at{gi}', tag='at')
                    at_tiles[gi] = at_g
                    a_tiles = []
                    for t in range(QT):
                        # interleave 2 attn@V matmuls of the previous unit
                        for _ in range(2):
                            if pend_i < len(pend):
                                pend[pend_i]()
                                pend_i += 1
                        blo = 896 - 128 * t
                        qsl = slice(t * 128, (t + 1) * 128)
                        # pos band matmuls -> 512-wide PSUM chunks, evicted to SBUF
                        pb_t = BP.tile([128, BAND], bf16, name='Pb', tag='Pb')
                        for ci, (c0, n) in enumerate(((0, 512), (512, 512), (1024, 128))):
                            pps = SC.tile([128, 512], f32, name='P', tag='P')
                            nc.tensor.matmul(
                                pps[:, :n],
                                qpT[b][p][rows, qsl],
                                kpT[p][rows, blo + c0:blo + c0 + n],
                                start=True, stop=True)
                            if ci == 1:
                                nc.vector.tensor_copy(pb_t[:, c0:c0 + n], pps[:, :n])
                            else:
                                nc.scalar.copy(pb_t[:, c0:c0 + n], pps[:, :n])
                        # rel-shift: sheared SBUF->SBUF DMA (SWDGE)
                        src = pb_t[:]
                        sheared = src.__replace__(
                            ap=src.ap.__class__([[BAND - 1, 128], [1, S]]),
                            offset=127)
                        sh_t = SHP.tile([128, S], bf16, name='Sh', tag='Sh')
                        nc.gpsimd.dma_start(sh_t[:], sheared)
                        # content scores
                        cps = SC.tile([128, S], f32, name='C', tag='C')
                        for c in range(2):
                            csl = slice(c * 512, (c + 1) * 512)
                            nc.tensor.matmul(
                                cps[:, csl], qcT[b][p][rows, qsl], kTt[b][p][rows, csl],
                                start=True, stop=True)
                        # pos + content add on DVE (only engine with PSUM tensor ops)
                        sum_t = SUP.tile([128, S], f32, name='Su', tag='Su')
                        nc.vector.tensor_tensor(sum_t[:], cps[:], sh_t[:], ADD)
                        a_t = AP_.tile([128, S], bf16, name='A', tag='A', bufs=4)
                        den = SCP.tile([128, 1], f32, name='den', tag='den')
                        nc.scalar.activation(a_t[:], sum_t[:], Exp, accum_out=den[:])
                        rec = SCP.tile([128, 1], f32, name='rec', tag='rec')
                        nc.vector.reciprocal(rec[:], den[:])
                        as_t = AP_.tile([128, S], bf16, name='As', tag='As', bufs=9)
                        if t % 2 == 0:
                            nc.scalar.mul(as_t[:], a_t[:], rec[:])
                        else:
                            nc.vector.tensor_scalar_mul(as_t[:], a_t[:], rec[:])
                        a_tiles.append(as_t)
                    # batched xbar transposes (minimize DMA xbar-mode transitions)
                    for t in range(QT):
                        nc.sync.dma_start_transpose(at_g[:, :, t, :], a_tiles[t])
                    if gi % 2 == 1 and pend:
                        # drain any leftover attn@V matmuls, then evict
                        while pend_i < len(pend):
                            pend[pend_i]()
                            pend_i += 1
                        pend = []
                        emit_ov_evict(gi // 2 - 1)

                # tail attn@V (unit 3) still inside the score PSUM pool scope
                for mm in attn_mms(3):
                    mm()
                emit_ov_evict(3)

            # ---- out projection (scores PSUM pool closed; fresh banks) ----
            with tc.tile_pool(name='op', bufs=3, space='PSUM') as OP, \
                 tc.tile_pool(name='oev', bufs=3) as OEV:
                for b in range(NB):
                    for sc in range(2):
                        for eb in range(EC):
                            po = OP.tile([128, 512], f32, name='OP', tag='OP')
                            for p in range(PAIRS):
                                nc.tensor.matmul(
                                    po[:],
                                    woS[p][:, eb * 128:(eb + 1) * 128],
                                    oT[b][p][sc][:],
                                    start=(p == 0), stop=(p == PAIRS - 1))
                            ev = OEV.tile([128, 512], f32, name='oev', tag='oev')
                            nc.vector.tensor_copy(ev[:], po[:])
                            nc.scalar.dma_start(
                                outT[b].ap()[eb * 128:(eb + 1) * 128,
                                             sc * 512:(sc + 1) * 512],
                                ev[:])

    nc.compile()
    return nc


def _prep_inputs(inputs):
    """Full inputs -> list of per-core input dicts (host-side shard + layout)."""
    bf = ml_dtypes.bfloat16
    q = np.asarray(inputs['query'], np.float32)
    k = np.asarray(inputs['key'], np.float32)
    v = np.asarray(inputs['value'], np.float32)
    pe = np.asarray(inputs['pe'], np.float32)
    w_q = np.asarray(inputs['w_q'], np.float32)
    w_k = np.asarray(inputs['w_k'], np.float32)
    w_v = np.asarray(inputs['w_v'], np.float32)
    w_kp = np.asarray(inputs['w_k_pos'], np.float32)
    cb = np.asarray(inputs['content_bias'], np.float32)
    pb = np.asarray(inputs['pos_bias'], np.float32)
    w_out = np.asarray(inputs['w_out'], np.float32)

    M = 2 * S - 1
    lower = pe.shape[0] // 2 - S + 1
    pe_sl = pe[lower:lower + M]                     # [2047, E]
    peT = np.zeros((E, MT), np.float32)
    peT[:, :M] = pe_sl.T

    qTb = [np.ascontiguousarray(q[:, b, :].T).astype(bf) for b in range(B)]
    kTb = [np.ascontiguousarray(k[:, b, :].T).astype(bf) for b in range(B)]
    vTb = [np.ascontiguousarray(v[:, b, :].T).astype(bf) for b in range(B)]
    peTb = peT.astype(bf)

    in_maps = []
    for c in range(N_CORES):
        bg, hg = divmod(c, 4)
        hs = hg * NH
        ch = slice(hs * D, (hs + NH) * D)           # this core's 256 channels
        m = {
            'peT': peTb,
            'wqT': np.ascontiguousarray((SCALING * w_q[ch, :]).T).astype(bf),
            'wkT': np.ascontiguousarray(w_k[ch, :].T).astype(bf),
            'wvT': np.ascontiguousarray(w_v[ch, :].T).astype(bf),
            'wpT': np.ascontiguousarray(w_kp[ch, :].T).astype(bf),
            'woT': np.ascontiguousarray(w_out[:, ch].T).astype(bf),
            'cb': (SCALING * cb[hs:hs + NH].reshape(CH, 1)).astype(np.float32),
            'pb': (SCALING * pb[hs:hs + NH].reshape(CH, 1)).astype(np.float32),
        }
        for i in range(NB):
            b = 2 * bg + i
            m[f'qT{i}'] = qTb[b]
            m[f'kT{i}'] = kTb[b]
            m[f'vT{i}'] = vTb[b]
        in_maps.append(m)
    return in_maps


def kernel(**inputs):
    from concourse import bass_utils

    if 'nc' not in _cache:
        _cache['nc'] = _build()
    nc = _cache['nc']

    in_maps = _prep_inputs(inputs)
    res = bass_utils.run_bass_kernel_spmd(nc, in_maps, core_ids=list(range(N_CORES)))
    _cache['last_results'] = res

    b_out = np.asarray(inputs['b_out'], np.float32)
    out = np.empty((S, B, E), np.float32)
    for b in range(B):
        bg, i = divmod(b, NB)
        acc = res.results[4 * bg][f'outT{i}'].copy()
        for hg in range(1, 4):
            acc += res.results[4 * bg + hg][f'outT{i}']
        out[:, b, :] = acc.T + b_out
    return out


# revision 26
# speedup vs baseline: 2.6398x; 1.0429x over previous
"""MultiHeadRelativeAttention (Transformer-XL style) on 8 Trainium2 NeuronCores.

Sharding: 2 batches x 4 heads per core (core c -> batches {2*(c//4), 2*(c//4)+1},
heads [4*(c%4), 4*(c%4)+4)). Each core computes a partial out-projection over its
256 input channels for each of its 2 batches; the host sums 4 partials per batch.

Pipeline (vs. the previous version):
  - The Transformer-XL relative shift stays a sheared SBUF->SBUF DMA
    (access pattern [[BAND-1,128],[1,1024]] offset 127 over a [128,BAND] band).
  - pos+content add moved off the PE (was an identity matmul) onto Pool/DVE.
  - softmax normalization deferred: exp(A) flows unnormalized into attn@V;
    1/den is applied per-column during the PSUM->SBUF eviction of attn@V
    using a transposed+broadcast reciprocal tile (xbar transpose of packed recs).
  - attn@V matmuls of head-unit u are interleaved into the score matmuls of
    unit u+1 so the PE never waits on the shear/exp/transpose chain.
  - k_pos projection computed once per core for 4 heads (halved vs 8-head shard).
"""
import sys

sys.path.insert(0, '/opt/trn_rl_repo')

import numpy as np
import ml_dtypes

S = 1024          # seq len (query == key)
B = 4             # batch
E = 1024          # embed dim
H = 16            # total heads
D = 64            # head dim
NB = 2            # batches per core
NH = 4            # heads per core
PAIRS = NH // 2   # head pairs per core (2 heads packed per 128 partitions)
CH = NH * D       # channels per core (256)
MT = 2048         # padded positional length (2*S-1 = 2047 valid)
BAND = S + 128    # per-q-tile band width in m
QT = S // 128     # q tiles
KC = S // 128     # k chunks of 128
EC = E // 128     # embed chunks
SCALING = D ** -0.5
N_CORES = 8

_cache = {}


def _build():
    import concourse.bass as bass
    from concourse import bacc
    import concourse.mybir as mybir
    from concourse.tile import TileContext

    bf16 = mybir.dt.bfloat16
    f32 = mybir.dt.float32
    Exp = mybir.ActivationFunctionType.Exp
    ADD = mybir.AluOpType.add
    MULT = mybir.AluOpType.mult

    nc = bacc.Bacc("TRN2", debug=False, num_devices=N_CORES)

    def din(name, shape, dt=bf16):
        return nc.dram_tensor(name, shape, dt, kind='ExternalInput')

    qT = [din(f'qT{b}', [E, S]) for b in range(NB)]
    kT = [din(f'kT{b}', [E, S]) for b in range(NB)]
    vT = [din(f'vT{b}', [E, S]) for b in range(NB)]
    peT = din('peT', [E, MT])
    wqT = din('wqT', [E, CH])
    wkT = din('wkT', [E, CH])
    wvT = din('wvT', [E, CH])
    wpT = din('wpT', [E, CH])
    woT = din('woT', [CH, E])
    cbv = din('cb', [CH, 1], f32)
    pbv = din('pb', [CH, 1], f32)
    outT = [nc.dram_tensor(f'outT{b}', [E, S], f32, kind='ExternalOutput')
            for b in range(NB)]

    with TileContext(nc) as tc:
        with tc.tile_pool(name='persist', bufs=1) as PERS:

            # ---- persistent tiles ----
            qcT = [[PERS.tile([128, S], bf16, name=f'qcT{b}{p}', tag=f'qcT{b}{p}')
                    for p in range(PAIRS)] for b in range(NB)]
            qpT = [[PERS.tile([128, S], bf16, name=f'qpT{b}{p}', tag=f'qpT{b}{p}')
                    for p in range(PAIRS)] for b in range(NB)]
            kTt = [[PERS.tile([128, S], bf16, name=f'kTt{b}{p}', tag=f'kTt{b}{p}')
                    for p in range(PAIRS)] for b in range(NB)]
            kpT = [PERS.tile([128, MT], bf16, name=f'kpT{p}', tag=f'kpT{p}')
                   for p in range(PAIRS)]
            vS = [[PERS.tile([128, CH], bf16, name=f'vS{b}{k}', tag=f'vS{b}{k}')
                   for k in range(KC)] for b in range(NB)]
            woS = [PERS.tile([128, E], bf16, name=f'woS{p}', tag=f'woS{p}')
                   for p in range(PAIRS)]
            # attn@V output (normalized), bf16, [2heads*64d x 512q] per (b,p,sc)
            oT = [[[PERS.tile([128, 512], bf16, name=f'oT{b}{p}{sc}', tag=f'oT{b}{p}{sc}')
                    for sc in range(2)] for p in range(PAIRS)] for b in range(NB)]
            cbS = [PERS.tile([128, 1], f32, name=f'cbS{p}', tag=f'cbS{p}')
                   for p in range(PAIRS)]
            pbS = [PERS.tile([128, 1], f32, name=f'pbS{p}', tag=f'pbS{p}')
                   for p in range(PAIRS)]

            # weight/bias loads on the Act HWDGE queue (SP queue handles the
            # big activations + pe table + transposes)
            for p in range(PAIRS):
                nc.scalar.dma_start(cbS[p][:], cbv.ap()[p * 128:(p + 1) * 128, :])
                nc.scalar.dma_start(pbS[p][:], pbv.ap()[p * 128:(p + 1) * 128, :])
            for p in range(PAIRS):
                nc.scalar.dma_start(woS[p][:], woT.ap()[p * 128:(p + 1) * 128, :])

            # ---- projections ----
            with tc.tile_pool(name='pp', bufs=2, space='PSUM') as PP, \
                 tc.tile_pool(name='stage', bufs=8) as STG:
                win_q = [STG.tile([128, CH], bf16, name='winq', tag='winq') for _ in range(EC)]
                win_k = [STG.tile([128, CH], bf16, name='wink', tag='wink') for _ in range(EC)]
                win_v = [STG.tile([128, CH], bf16, name='winv', tag='winv') for _ in range(EC)]
                win_p = [STG.tile([128, CH], bf16, name='winp', tag='winp') for _ in range(EC)]
                for ec in range(EC):
                    esl = slice(ec * 128, (ec + 1) * 128)
                    nc.scalar.dma_start(win_q[ec][:], wqT.ap()[esl, :])
                    nc.scalar.dma_start(win_k[ec][:], wkT.ap()[esl, :])
                    nc.scalar.dma_start(win_v[ec][:], wvT.ap()[esl, :])
                    nc.scalar.dma_start(win_p[ec][:], wpT.ap()[esl, :])

                # q/k projections per batch -> qcT/qpT (biased) and kTt
                xq, xk, xv = {}, {}, {}
                for b in range(NB):
                    xq[b] = [STG.tile([128, S], bf16, name='xin', tag='xin') for _ in range(EC)]
                    for ec in range(EC):
                        nc.sync.dma_start(xq[b][ec][:], qT[b].ap()[ec * 128:(ec + 1) * 128, :])
                    for p in range(PAIRS):
                        ps = PP.tile([128, S], f32, name='qps', tag='qps')
                        for c in range(2):
                            for ec in range(EC):
                                nc.tensor.matmul(
                                    ps[:, c * 512:(c + 1) * 512],
                                    win_q[ec][:, p * 128:(p + 1) * 128],
                                    xq[b][ec][:, c * 512:(c + 1) * 512],
                                    start=(ec == 0), stop=(ec == EC - 1))
                        nc.vector.tensor_scalar_add(qcT[b][p][:], ps[:], cbS[p][:])
                        nc.vector.tensor_scalar_add(qpT[b][p][:], ps[:], pbS[p][:])

                    xk[b] = [STG.tile([128, S], bf16, name='xin', tag='xin') for _ in range(EC)]
                    for ec in range(EC):
                        nc.sync.dma_start(xk[b][ec][:], kT[b].ap()[ec * 128:(ec + 1) * 128, :])
                    for p in range(PAIRS):
                        ps = PP.tile([128, S], f32, name='qps', tag='qps')
                        for c in range(2):
                            for ec in range(EC):
                                nc.tensor.matmul(
                                    ps[:, c * 512:(c + 1) * 512],
                                    win_k[ec][:, p * 128:(p + 1) * 128],
                                    xk[b][ec][:, c * 512:(c + 1) * 512],
                                    start=(ec == 0), stop=(ec == EC - 1))
                        nc.vector.tensor_copy(kTt[b][p][:], ps[:])

                # k_pos projection -> kpT (once per core, 4 heads)
                for mc in range(MT // 512):
                    pein = [STG.tile([128, 512], bf16, name='pein', tag='pein') for _ in range(EC)]
                    for ec in range(EC):
                        nc.sync.dma_start(
                            pein[ec][:],
                            peT.ap()[ec * 128:(ec + 1) * 128, mc * 512:(mc + 1) * 512])
                    for p in range(PAIRS):
                        ps = PP.tile([128, 512], f32, name='kps', tag='kps')
                        for ec in range(EC):
                            nc.tensor.matmul(
                                ps[:], win_p[ec][:, p * 128:(p + 1) * 128], pein[ec][:],
                                start=(ec == 0), stop=(ec == EC - 1))
                        nc.scalar.copy(kpT[p][:, mc * 512:(mc + 1) * 512], ps[:])

                # v projection -> vS[b][kt] = [128 k x (h,d) 256]
                for b in range(NB):
                    xv[b] = [STG.tile([128, S], bf16, name='xin', tag='xin') for _ in range(EC)]
                    for ec in range(EC):
                        nc.sync.dma_start(xv[b][ec][:], vT[b].ap()[ec * 128:(ec + 1) * 128, :])
                    for kt in range(KC):
                        ps = PP.tile([128, CH], f32, name='vps', tag='vps')
                        for ec in range(EC):
                            nc.tensor.matmul(
                                ps[:], xv[b][ec][:, kt * 128:(kt + 1) * 128], win_v[ec][:],
                                start=(ec == 0), stop=(ec == EC - 1))
                        if kt % 2 == 0:
                            nc.vector.tensor_copy(vS[b][kt][:], ps[:])
                        else:
                            nc.scalar.copy(vS[b][kt][:], ps[:])

            # ---- scores + attention ----
            groups = [(b, p, h01) for b in range(NB) for p in range(PAIRS)
                      for h01 in range(2)]
            units = [(b, p) for b in range(NB) for p in range(PAIRS)]

            with tc.tile_pool(name='sc', bufs=2, space='PSUM') as SC, \
                 tc.tile_pool(name='bandp', bufs=3) as BP, \
                 tc.tile_pool(name='shp', bufs=4) as SHP, \
                 tc.tile_pool(name='sump', bufs=3) as SUP, \
                 tc.tile_pool(name='ap', bufs=4) as AP_, \
                 tc.tile_pool(name='atp', bufs=4) as ATP, \
                 tc.tile_pool(name='scp', bufs=4) as SCP:

                at_tiles = {}    # group idx -> at tile
                ops_tile = {}    # unit idx -> attn@V PSUM tile

                def attn_mms(u):
                    """Build the 32 attn@V matmul closures for unit u.

                    h01=0 matmuls first: they only need at[2u] (transposed one
                    group earlier), so the first pops never wait on at[2u+1]."""
                    b, p = units[u]
                    ops = SC.tile([128, S], f32, name=f'O{u}', tag='O', bufs=1)
                    ops_tile[u] = ops
                    atg = [at_tiles[2 * u], at_tiles[2 * u + 1]]
                    mms = []
                    for h01 in range(2):
                        hh = 2 * p + h01
                        for kc in range(KC):
                            for sc in range(2):
                                mms.append(lambda kc=kc, h01=h01, hh=hh, sc=sc:
                                    nc.tensor.matmul(
                                        ops[64 * h01:64 * h01 + 64,
                                            512 * sc:512 * sc + 512],
                                        vS[b][kc][:, hh * 64:hh * 64 + 64],
                                        atg[h01][:, kc, 4 * sc:4 * sc + 4, :],
                                        start=(kc == 0), stop=(kc == KC - 1)))
                    mms.append(lambda u=u: emit_ov_evict(u))
                    return mms

                def emit_ov_evict(u):
                    b, p = units[u]
                    ops = ops_tile[u]
                    nc.vector.tensor_copy(oT[b][p][0][:], ops[:, 0:512])
                    nc.scalar.copy(oT[b][p][1][:], ops[:, 512:1024])

                # Software-pipelined flat slot loop: the content matmul (and the
                # add/exp chain behind it) for tile s runs at slot s+2, so the
                # sheared pos band has already landed when the DVE add fires and
                # the content PSUM tile is freed immediately.
                DELAY = 2
                a_tiles = {}     # group -> list of normalized bf16 tiles
                pend = []
                pend_i = 0

                def pos_stage(s):
                    gi, t = divmod(s, QT)
                    b, p, h01 = groups[gi]
                    rows = slice(64 * h01, 64 * h01 + 64)
                    blo = 896 - 128 * t
                    qsl = slice(t * 128, (t + 1) * 128)
                    pb_t = BP.tile([128, BAND], bf16, name='Pb', tag='Pb')
                    for ci, (c0, n) in enumerate(((0, 512), (512, 512), (1024, 128))):
                        pps = SC.tile([128, 512], f32, name='P', tag='P')
                        nc.tensor.matmul(
                            pps[:, :n],
                            qpT[b][p][rows, qsl],
                            kpT[p][rows, blo + c0:blo + c0 + n],
                            start=True, stop=True)
                        if ci == 2 or (ci == 1 and t % 2 == 0):
                            nc.vector.tensor_copy(pb_t[:, c0:c0 + n], pps[:, :n])
                        else:
                            nc.scalar.copy(pb_t[:, c0:c0 + n], pps[:, :n])
                    # rel-shift: sheared SBUF->SBUF DMA (SWDGE)
                    src = pb_t[:]
                    sheared = src.__replace__(
                        ap=src.ap.__class__([[BAND - 1, 128], [1, S]]),
                        offset=127)
                    sh_t = SHP.tile([128, S], bf16, name='Sh', tag='Sh')
                    nc.gpsimd.dma_start(sh_t[:], sheared)
                    return sh_t

                def content_stage(s, sh_t):
                    gi, t = divmod(s, QT)
                    b, p, h01 = groups[gi]
                    rows = slice(64 * h01, 64 * h01 + 64)
                    qsl = slice(t * 128, (t + 1) * 128)
                    cps = SC.tile([128, S], f32, name='C', tag='C')
                    for c in range(2):
                        csl = slice(c * 512, (c + 1) * 512)
                        nc.tensor.matmul(
                            cps[:, csl], qcT[b][p][rows, qsl], kTt[b][p][rows, csl],
                            start=True, stop=True)
                    # pos + content add on DVE (only engine with PSUM tensor ops)
                    sum_t = SUP.tile([128, S], f32, name='Su', tag='Su')
                    nc.vector.tensor_tensor(sum_t[:], cps[:], sh_t[:], ADD)
                    a_t = AP_.tile([128, S], bf16, name='A', tag='A', bufs=4)
                    den = SCP.tile([128, 1], f32, name='den', tag='den')
                    nc.scalar.activation(a_t[:], sum_t[:], Exp, accum_out=den[:])
                    rec = SCP.tile([128, 1], f32, name='rec', tag='rec')
                    nc.vector.reciprocal(rec[:], den[:])
                    as_t = AP_.tile([128, S], bf16, name='As', tag='As', bufs=9)
                    nc.vector.tensor_scalar_mul(as_t[:], a_t[:], rec[:])
                    a_tiles.setdefault(gi, []).append(as_t)
                    if t == QT - 1:
                        # group complete: batched xbar transposes
                        at_g = ATP.tile([128, KC, QT, 128], bf16,
                                        name=f'at{gi}', tag='at')
                        at_tiles[gi] = at_g
                        for tt in range(QT):
                            nc.sync.dma_start_transpose(
                                at_g[:, :, tt, :], a_tiles[gi][tt])
                        return gi
                    return None

                sh_q = []
                for s in range(QT * len(groups) + DELAY):
                    # interleave attn@V matmuls of the previous unit
                    for _ in range(2):
                        if pend_i < len(pend):
                            pend[pend_i]()
                            pend_i += 1
                    if s < QT * len(groups):
                        sh_q.append(pos_stage(s))
                    if s >= DELAY:
                        done_gi = content_stage(s - DELAY, sh_q[s - DELAY])
                        if done_gi is not None and done_gi % 2 == 1:
                            # both groups of unit u transposed: queue its attn@V
                            while pend_i < len(pend):
                                pend[pend_i]()
                                pend_i += 1
                            pend = attn_mms(done_gi // 2)
                            pend_i = 0

                # drain the last unit's attn@V
                while pend_i < len(pend):
                    pend[pend_i]()
                    pend_i += 1

            # ---- out projection (scores PSUM pool closed; fresh banks) ----
            with tc.tile_pool(name='op', bufs=3, space='PSUM') as OP, \
                 tc.tile_pool(name='oev', bufs=3) as OEV:
                for b in range(NB):
                    for sc in range(2):
                        for eb in range(EC):
                            po = OP.tile([128, 512], f32, name='OP', tag='OP')
                            for p in range(PAIRS):
                                nc.tensor.matmul(
                                    po[:],
                                    woS[p][:, eb * 128:(eb + 1) * 128],
                                    oT[b][p][sc][:],
                                    start=(p == 0), stop=(p == PAIRS - 1))
                            ev = OEV.tile([128, 512], f32, name='oev', tag='oev')
                            nc.vector.tensor_copy(ev[:], po[:])
                            nc.scalar.dma_start(
                                outT[b].ap()[eb * 128:(eb + 1) * 128,
                                             sc * 512:(sc + 1) * 512],
                                ev[:])

    nc.compile()
    return nc


def _prep_inputs(inputs):
    """Full inputs -> list of per-core input dicts (host-side shard + layout)."""
    bf = ml_dtypes.bfloat16
    q = np.asarray(inputs['query'], np.float32)
    k = np.asarray(inputs['key'], np.float32)
    v = np.asarray(inputs['value'], np.float32)
    pe = np.asarray(inputs['pe'], np.float32)
    w_q = np.asarray(inputs['w_q'], np.float32)
    w_k = np.asarray(inputs['w_k'], np.float32)
    w_v = np.asarray(inputs['w_v'], np.float32)
    w_kp = np.asarray(inputs['w_k_pos'], np.float32)
    cb = np.asarray(inputs['content_bias'], np.float32)
    pb = np.asarray(inputs['pos_bias'], np.float32)
    w_out = np.asarray(inputs['w_out'], np.float32)

    M = 2 * S - 1
    lower = pe.shape[0] // 2 - S + 1
    pe_sl = pe[lower:lower + M]                     # [2047, E]
    peT = np.zeros((E, MT), np.float32)
    peT[:, :M] = pe_sl.T

    qTb = [np.ascontiguousarray(q[:, b, :].T).astype(bf) for b in range(B)]
    kTb = [np.ascontiguousarray(k[:, b, :].T).astype(bf) for b in range(B)]
    vTb = [np.ascontiguousarray(v[:, b, :].T).astype(bf) for b in range(B)]
    peTb = peT.astype(bf)

    in_maps = []
    for c in range(N_CORES):
        bg, hg = divmod(c, 4)
        hs = hg * NH
        ch = slice(hs * D, (hs + NH) * D)           # this core's 256 channels
        m = {
            'peT': peTb,
            'wqT': np.ascontiguousarray((SCALING * w_q[ch, :]).T).astype(bf),
            'wkT': np.ascontiguousarray(w_k[ch, :].T).astype(bf),
            'wvT': np.ascontiguousarray(w_v[ch, :].T).astype(bf),
            'wpT': np.ascontiguousarray(w_kp[ch, :].T).astype(bf),
            'woT': np.ascontiguousarray(w_out[:, ch].T).astype(bf),
            'cb': (SCALING * cb[hs:hs + NH].reshape(CH, 1)).astype(np.float32),
            'pb': (SCALING * pb[hs:hs + NH].reshape(CH, 1)).astype(np.float32),
        }
        for i in range(NB):
            b = 2 * bg + i
            m[f'qT{i}'] = qTb[b]
            m[f'kT{i}'] = kTb[b]
            m[f'vT{i}'] = vTb[b]
        in_maps.append(m)
    return in_maps


def kernel(**inputs):
    from concourse import bass_utils

    if 'nc' not in _cache:
        _cache['nc'] = _build()
    nc = _cache['nc']

    in_maps = _prep_inputs(inputs)
    res = bass_utils.run_bass_kernel_spmd(nc, in_maps, core_ids=list(range(N_CORES)))
    _cache['last_results'] = res

    b_out = np.asarray(inputs['b_out'], np.float32)
    out = np.empty((S, B, E), np.float32)
    for b in range(B):
        bg, i = divmod(b, NB)
        acc = res.results[4 * bg][f'outT{i}'].copy()
        for hg in range(1, 4):
            acc += res.results[4 * bg + hg][f'outT{i}']
        out[:, b, :] = acc.T + b_out
    return out
